# revision 1
# baseline (speedup 1.0000x reference)
"""ChainCRF loss kernel for Trainium2 (8 NeuronCores, data-parallel over batch).

Math: the CRF forward recurrence
    part_t[j] = em[t, j] + logsumexp_i(part_{t-1}[i] + trans[i, j])
is computed in exp space:  V_t = E_t * (ET^T @ V_{t-1}),  E = exp(em - 8*ln2),
ET = exp(trans).  The per-step 2^-8 rescale keeps values in range; the absorbed
scale count is restored on the host.

Each of the 4 sequences per core is split into G=64 time-chunks of length 4,
processed as two half-phases (g<32 sources only even bt-tiles, so phase A
starts while the odd-tile gathers still run). All (batch, chunk) columns of a
half advance together through R=8 rounds of one [128,128] bf16 matmul
(stationary exp(trans)) + one elementwise multiply.
Chunks g>=1 start K=4 rounds early from a uniform vector: the Perron
contraction of the positive chain matrices makes the state direction converge,
so a chunk's state equals the true forward state up to a per-column scalar.
Those scalars are recovered on the host by matching each chunk's log-state at
its boundary time (snapshot after round K-1) against the previous chunk's
final state, averaging over the 128 labels.

tgt_energy = sum_t trans[prev_t, tgt_t] + em[t, tgt_t] is computed on device:
G2 = OHpT.T @ trans (OHpT is the host-encoded one-hot of prev, an index
relayout), X = em + G2, then a fused (iota == tgt) * X select-and-accumulate
per 128-row tile.

Every instruction is kept to at most ONE semaphore wait (this walrus build
rejects more): producers are grouped per engine, consumers ordered so earlier
waits cover later deps, small "observer" ops absorb extra cross-engine waits,
the chain writes a fresh state tile per round (same-engine WAW on DVE emits
waits), and the Tile end-of-kernel drain is split into single-wait drains.
"""

import numpy as np

# problem dims (hardcoded per contract)
B, L, VOCAB, C = 32, 256, 50000, 128
NCORES = 8
BPC = B // NCORES      # 4 sequences per core
G = 64                 # chunks per sequence
CL = L // G            # 8 steps per chunk
K = 4                  # burn-in rounds (K=CL makes each chunk's burn-in
                       # window coincide with the previous chunk's real window,
                       # so matching residuals cancel)
R = K + CL             # 8 rounds
F = BPC * G            # 128 chain columns per core
NT = (BPC * L) // 128  # 8 gather tiles of 128 rows per core
PAIRED_GATHER = False  # HW dest mapping of [128,2] offset APs differs from
                       # CoreSim (odd tiles land wrong); keep 8 single gathers
LN2 = 0.6931471805599453
SBITS = 8              # per-step rescale = 2^-SBITS
OUT_W = 2 * F + NT     # out: [128, logV(F) | logSnap(F) | tgtsum(8)]


def _make_tc_class():
    import concourse.tile as tile
    from concourse.vector_clock import ScopedClock, VectorClock

    class SingleWaitTC(tile.TileContext):
        """TileContext whose end-of-kernel drain is split into single-wait
        sync-engine drains (this walrus rejects >1 wait per instruction)."""

        def _drain_and_barrier(self, tick_clock, wait_clock):
            nc = self.nc
            gc = tick_clock.global_clock
            n = len(gc)
            for p in range(n):
                t = gc[p]
                if t <= 0:
                    continue
                vec = [0] * n
                vec[p] = t
                nop = nc.sync.drain()
                wait_clock.add_sem_waits(
                    nop.ins, ScopedClock({None: VectorClock(vec)}))
            # per-proc drains above already waited on everything (including
            # the output DMA queues), so outputs are in DRAM; skip the EVSEM
            # butterfly barrier (~5-7us) and sem clears entirely — each
            # kernel() call loads a fresh NEFF, so semaphores start from
            # their load-time values
            nc.sync.drain()
            popped = nc._tile_sem_poison_stack.pop()
            assert popped is self._sem_poison

    return SingleWaitTC


def _build():
    import concourse.bass as bass
    import concourse.tile as tile
    from concourse import mybir

    f32 = mybir.dt.float32
    bf16 = mybir.dt.bfloat16
    i32 = mybir.dt.int32
    Alu = mybir.AluOpType
    Act = mybir.ActivationFunctionType

    from concourse.masks import make_identity

    nc = bass.Bass("TRN2", debug=False)

    table_d = nc.dram_tensor("table", [VOCAB, C], f32, kind="ExternalInput").ap()
    # packed inputs: keep total HWDGE DMA count <= queue count, else DMAs
    # inherit a second (queue-ordering) wait and fail the 1-wait budget
    tt_d = nc.dram_tensor("tt", [128, 2 * NT], i32, kind="ExternalInput").ap()
    transx_d = nc.dram_tensor("transx", [C, C + 1], f32,
                              kind="ExternalInput").ap()
    ohpx_d = nc.dram_tensor("ohpx", [128, NT * 128 + C], bf16,
                            kind="ExternalInput").ap()
    out_d = nc.dram_tensor("out", [128, OUT_W], f32, kind="ExternalOutput").ap()

    def mkap(t_ap, offset, dims):
        # dims: list of [stride, count] free dims; partition dim prepended
        return bass.AP(t_ap.tensor, offset, [t_ap.ap[0]] + dims)

    TC = _make_tc_class()
    with TC(nc) as tc:
        with (
            tc.tile_pool(name="sb", bufs=1) as sb,
            tc.tile_pool(name="ps", bufs=1, space="PSUM") as psp,
        ):
            def st(shape, dt, nm):
                return sb.tile(shape, dt, name=nm, tag=nm)

            def pt(shape, dt, nm):
                return psp.tile(shape, dt, name=nm, tag=nm)

            # ---- input DMAs (3 total; tokens+targets first) ----
            tt_sb = st([128, 2 * NT], i32, "tt_sb")
            nc.sync.dma_start(tt_sb[:], tt_d)
            toks_sb = tt_sb[:, 0:NT]
            tgt_sb = tt_sb[:, NT:2 * NT]
            transx_sb = st([C, C + 1], f32, "transx_sb")
            nc.scalar.dma_start(transx_sb[:], transx_d)
            trans_sb = transx_sb[:, 0:C]
            tr127_sb = transx_sb[:, C:C + 1]
            ohpx_sb = st([128, NT * 128 + C], bf16, "ohpx_sb")
            nc.sync.dma_start(ohpx_sb[:], ohpx_d)
            ohp_sb = ohpx_sb[:, 0:NT * 128]
            transb_sb = ohpx_sb[:, NT * 128:NT * 128 + C]

            # ---- gpsimd: identity first (tiny), then gathers ASAP.
            # Even bt-tiles (t in [0,128)) first: they feed chunks g<32, so
            # chain phase A runs while the odd-tile gathers continue ----
            ident = st([128, 128], f32, "ident")
            make_identity(nc, ident[:])
            # junk/bias before the gathers: chain A's observers wait on these
            # Pool ticks, and anything emitted after the gathers lands ~21us.
            # Separate per-half window tensors: interval-based access tracking
            # would otherwise serialize chain A behind the B-half exps.
            GH = G // 2          # chunks per half
            FH = BPC * GH        # chain columns per half; col = s*FH + b*GH + g
            ewinA = st([128, R * FH], bf16, "ewinA")
            ewinB = st([128, R * FH], bf16, "ewinB")
            ewa = ewinA[:]
            ewb = ewinB[:]
            nc.gpsimd.memset(mkap(ewa, 0, [[GH, BPC], [FH, K]]), 2.0 ** -SBITS)
            bias_t = st([128, 1], f32, "bias_t")
            nc.gpsimd.memset(bias_t[:], -float(SBITS) * LN2)
            em_sb = st([128, NT * 128], f32, "em_sb")
            GORDER = [0, 2, 4, 6, 1, 3, 5, 7]
            for k in GORDER:
                nc.gpsimd.indirect_dma_start(
                    out=em_sb[:, k * 128:(k + 1) * 128],
                    out_offset=None,
                    in_=table_d,
                    in_offset=bass.IndirectOffsetOnAxis(
                        ap=toks_sb[:, k:k + 1], axis=0),
                )

            # ---- gpsimd prep (after gathers; order matters for coverage) ----
            iota_row_i = st([128, 128], i32, "iota_row_i")
            nc.gpsimd.iota(iota_row_i[:], pattern=[[1, 128]], base=0,
                           channel_multiplier=0)
            iota_row = st([128, 128], f32, "iota_row")
            nc.gpsimd.tensor_copy(iota_row[:], iota_row_i[:])
            tgt_f = st([128, NT], f32, "tgt_f")       # LAST Pool op
            nc.gpsimd.tensor_copy(tgt_f[:], tgt_sb)

            # ---- PE: observer, even-tile transposes; A-exps; then odd ----
            emT = pt([128, NT * 128], f32, "emT")
            x_ps = pt([128, NT * 128], f32, "x_ps")
            nc.tensor.transpose(out=x_ps[:, 0:128], in_=ident[:],
                                identity=ident[:])
            for k in [0, 2, 4, 6]:
                nc.tensor.transpose(
                    out=emT[:, k * 128:(k + 1) * 128],
                    in_=em_sb[:, k * 128:(k + 1) * 128],
                    identity=ident[:],
                )

            ET = st([C, C], bf16, "ET")
            nc.scalar.activation(ET[:], trans_sb, Act.Exp)
            ET127 = st([C, 1], f32, "ET127")
            nc.scalar.activation(ET127[:], tr127_sb, Act.Exp)
            VA = [st([128, FH], bf16, f"VA{s}") for s in range(R + 1)]
            VB = [st([128, FH], bf16, f"VB{s}") for s in range(R + 1)]
            one = nc.const_aps.aps[(f32, 1.0)]
            nc.scalar.activation(VA[0][:], one[:128].to_broadcast([128, FH]),
                                 Act.Copy)
            nc.scalar.activation(VB[0][:], one[:128].to_broadcast([128, FH]),
                                 Act.Copy)
            scra = st([128, 1], f32, "scra")
            nc.scalar.activation(scra[:1, 0:1], bias_t[:1, 0:1], Act.Copy)

            emt = emT[:]
            bias = bias_t[:]
            # half A: g in [1, GH)  <- src t = CL*(g-1)+s in [0, 128)
            # (emitted before the odd transposes so the program-order RAW dep
            # set is the even transposes only)
            for b in range(BPC):
                nc.scalar.activation(
                    mkap(ewa, b * GH + 1, [[FH, R], [1, GH - 1]]),
                    mkap(emt, b * L, [[1, R], [CL, GH - 1]]),
                    Act.Exp, bias=bias)
            # chunk0 real steps: s in [K+1, R) <- t = s-K in [1, CL)
            nc.scalar.activation(
                mkap(ewa, (K + 1) * FH, [[GH, BPC], [FH, CL - 1]]),
                mkap(emt, 1, [[L, BPC], [1, CL - 1]]),
                Act.Exp, bias=bias)
            # E0 at chunk0 col s=K (re-init source; last A-feeding exp)
            nc.scalar.activation(
                mkap(ewa, K * FH, [[GH, BPC]]),
                mkap(emt, 0, [[L, BPC]]),
                Act.Exp, bias=bias)

            # PE observer of the A-exps (E0 was the last one), so the odd
            # transposes' WAR on emT doesn't add a second wait
            nc.tensor.matmul(out=x_ps[0:1, 0:1],
                             lhsT=ewinA[:, K * FH:K * FH + 1],
                             rhs=ewinA[:, K * FH:K * FH + 1],
                             start=True, stop=True)
            for k in [1, 3, 5, 7]:
                nc.tensor.transpose(
                    out=emT[:, k * 128:(k + 1) * 128],
                    in_=em_sb[:, k * 128:(k + 1) * 128],
                    identity=ident[:],
                )
            # half B: g in [GH, G)  <- src t = CL*(g-1)+s in [124, 256)
            # ACT observer first: absorbs the ACT-self tick the first B-exp
            # would otherwise carry as a second wait
            scrb = st([128, 1], f32, "scrb")
            obs_act = nc.scalar.activation(
                scrb[:1, 0:1], ewinA[0:1, K * FH:K * FH + 1], Act.Copy)
            first_bexp = None
            for b in range(BPC):
                h = nc.scalar.activation(
                    mkap(ewb, b * GH, [[FH, R], [1, GH]]),
                    mkap(emt, b * L + CL * (GH - 1), [[1, R], [CL, GH]]),
                    Act.Exp, bias=bias)
                if first_bexp is None:
                    first_bexp = h
                    tile.add_dep_helper(h.ins, obs_act.ins, sync=False,
                                        reason="order ACT obs before B exps")

            # ---- output staging: [logVA |snapA |logVB |snapB] ----
            outsb = st([128, 2 * F], f32, "outsb")
            tgtout = st([128, NT], f32, "tgtout")
            scr = st([128, 4], f32, "scr")

            # ---- the chains (high priority; B can interleave into A) ----
            psA = pt([128, FH], f32, "psA")
            psB = pt([128, FH], f32, "psB")
            with tc.high_priority():
                # DVE observers for chain A (must share the chain's priority
                # or the scheduler orders the chain TTs before them)
                nc.vector.tensor_copy(scr[:1, 0:1], ewinA[:1, 0:1])
                obs_e = nc.vector.tensor_copy(scr[:1, 1:2],
                                              ewinA[:1, K * FH:K * FH + 1])
                tt0 = None
                for s in range(R):
                    nc.tensor.matmul(out=psA[:], lhsT=ET[:], rhs=VA[s][:],
                                     start=True, stop=True)
                    h = nc.vector.tensor_tensor(
                        out=VA[s + 1][:], in0=psA[:],
                        in1=ewinA[:, s * FH:(s + 1) * FH], op=Alu.mult)
                    if tt0 is None:
                        tt0 = h
                        tile.add_dep_helper(h.ins, obs_e.ins, sync=False,
                                            reason="order DVE obs before TTs")
                    if s == K - 1:
                        nc.scalar.activation(outsb[:, F:F + FH], VA[K][:],
                                             Act.Ln)
                    if s == K:
                        # re-init chunk-0 columns (b*GH) from true part0
                        nc.vector.tensor_scalar_mul(
                            mkap(VA[K + 1][:], 0, [[GH, BPC]]),
                            mkap(ewa, K * FH, [[GH, BPC]]),
                            ET127[:],
                        )
                nc.scalar.activation(outsb[:, 0:FH], VA[R][:], Act.Ln)
                # observer: absorb the B-half exps' ACT tick before B's TTs
                nc.vector.tensor_copy(
                    scr[:1, 2:3], ewinB[:1, 3 * GH:3 * GH + 1])
                for s in range(R):
                    nc.tensor.matmul(out=psB[:], lhsT=ET[:], rhs=VB[s][:],
                                     start=True, stop=True)
                    nc.vector.tensor_tensor(
                        out=VB[s + 1][:], in0=psB[:],
                        in1=ewinB[:, s * FH:(s + 1) * FH], op=Alu.mult)
                    if s == K - 1:
                        nc.scalar.activation(outsb[:, F + FH:2 * F], VB[K][:],
                                             Act.Ln)
                nc.scalar.activation(outsb[:, FH:F], VB[R][:], Act.Ln)

            # A block out as soon as chain A is done (overlaps chain B)
            nc.sync.dma_start(out_d[:, 0:F], outsb[:, 0:F])

            # ---- tgt energy ----
            # G2 matmuls; a dummy matmul reading only ohp_sb absorbs its DMA
            # wait so each G2 matmul carries at most one (transb's queue)
            nc.tensor.matmul(out=x_ps[:, 0:128], lhsT=ohp_sb[:, 0:128],
                             rhs=ohp_sb[:, 0:128], start=True, stop=True)
            for k in range(NT):
                nc.tensor.matmul(
                    out=x_ps[:, k * 128:(k + 1) * 128],
                    lhsT=ohp_sb[:, k * 128:(k + 1) * 128],
                    rhs=transb_sb,
                    start=True, stop=True,
                )
            # DVE observers: per-queue gather sems + Pool (tgt_f covers iota)
            scr8 = st([128, NT], f32, "scr8")
            for k in range(NT):
                nc.vector.tensor_copy(scr8[:1, k:k + 1],
                                      em_sb[:1, k * 128:k * 128 + 1])
            nc.vector.tensor_copy(scr[:1, 3:4], tgt_f[:1, 0:1])
            xs = st([128, NT * 128], f32, "xs")
            sel = st([128, NT * 128], f32, "sel")
            for k in range(NT):
                sl = slice(k * 128, (k + 1) * 128)
                nc.vector.tensor_tensor(out=xs[:, sl], in0=em_sb[:, sl],
                                        in1=x_ps[:, sl], op=Alu.add)
            for k in range(NT):
                sl = slice(k * 128, (k + 1) * 128)
                nc.vector.scalar_tensor_tensor(
                    out=sel[:, sl],
                    in0=iota_row[:],
                    scalar=tgt_f[:, k:k + 1],
                    in1=xs[:, sl],
                    op0=Alu.is_equal,
                    op1=Alu.mult,
                    accum_out=tgtout[:, k:k + 1],
                )

            nc.sync.dma_start(out_d[:, F:2 * F], outsb[:, F:2 * F])
            nc.sync.dma_start(out_d[:, 2 * F:OUT_W], tgtout[:])

    return nc


def _host_prep(tokens, target):
    """Per-core input maps. Index tensors are laid out column-major per
    128-row tile: arr[p, k] = flat[k*128 + p], flat index bt = b*L + t.
    ohp is the one-hot relayout of prev: ohp[i, bt] = (prev[bt] == i)."""
    import ml_dtypes
    bf16 = ml_dtypes.bfloat16
    tokens = np.ascontiguousarray(tokens, dtype=np.int32)
    target = np.ascontiguousarray(target, dtype=np.int32)
    prev = np.concatenate(
        [np.full((B, 1), C - 1, np.int32), target[:, :-1]], axis=1)
    iota = np.arange(C, dtype=np.int32)

    def cols(a):  # [BPC, L] -> [128, NT]
        return a.reshape(-1).reshape(NT, 128).T

    maps = []
    for c in range(NCORES):
        bs = slice(c * BPC, (c + 1) * BPC)
        pv = prev[bs].reshape(-1)  # [1024], bt order
        ohp = (pv[None, :] == iota[:, None]).astype(bf16)  # [128, 1024]
        maps.append({
            "tt": np.ascontiguousarray(
                np.concatenate([cols(tokens[bs]), cols(target[bs])], axis=1)),
            "ohpx": None,  # filled in _run (needs transb)
            "_ohp": ohp,
        })
    return maps


def _combine(outs):
    """Stitch chunk states into per-batch loss. outs: list of [128, OUT_W]."""
    loss = np.empty(B, np.float64)
    sc = SBITS * LN2
    endcnt = np.full(G, R, np.float64)
    endcnt[0] = CL
    for c in range(NCORES):
        o = outs[c].astype(np.float64)
        FH = F // 2
        GH = G // 2
        lv = np.concatenate([o[:, 0:FH].reshape(C, BPC, GH),
                             o[:, FH:F].reshape(C, BPC, GH)], axis=2)
        ls = np.concatenate([o[:, F:F + FH].reshape(C, BPC, GH),
                             o[:, F + FH:2 * F].reshape(C, BPC, GH)], axis=2)
        tg = o[:, 2 * F:2 * F + NT]
        for bl in range(BPC):
            e = 0.0
            for g in range(1, G):
                d = (ls[:, bl, g] + K * sc) - (lv[:, bl, g - 1] + endcnt[g - 1] * sc)
                e += d.mean()
            part = lv[:, bl, G - 1] + endcnt[G - 1] * sc - e
            m = part.max()
            logz = np.log(np.exp(part - m).sum()) + m
            # tiles 2*bl, 2*bl+1 hold this sequence's bt rows
            tgt_e = tg[:, 2 * bl].sum() + tg[:, 2 * bl + 1].sum()
            loss[c * BPC + bl] = logz - tgt_e
    return loss.astype(np.float32)


_LDW_PATCHED = False


def _patch_ldw_opt():
    """walrus --enable-ldw-opt=true rejects the standalone InstLdweights that
    bass emits for repeated bf16 stationary weights, and G2 matmuls can
    interleave with chain matmuls on the PE, so reload elision is unsafe
    anyway. No-op."""
    return


def _run(inputs, trace=False):
    from concourse import bass_utils

    tokens = np.asarray(inputs["tokens"])
    target = np.asarray(inputs["target"])
    table = np.ascontiguousarray(np.asarray(inputs["state_table"], np.float32))
    trans = np.ascontiguousarray(np.asarray(inputs["trans_matrix"], np.float32))

    _patch_ldw_opt()
    nc = _build()
    maps = _host_prep(tokens, target)
    import ml_dtypes
    transb = trans.astype(ml_dtypes.bfloat16)
    transx = np.ascontiguousarray(
        np.concatenate([trans, trans[C - 1:C, :].T], axis=1))
    for m in maps:
        ohp = m.pop("_ohp")
        m["ohpx"] = np.ascontiguousarray(
            np.concatenate([ohp, transb], axis=1))
        m["table"] = table
        m["transx"] = transx

    res = bass_utils.run_bass_kernel_spmd(
        nc, maps, core_ids=list(range(NCORES)), trace=trace)
    loss = _combine([r["out"] for r in res.results])
    return loss, res


def kernel(**inputs):
    loss, _ = _run(inputs, trace=False)
    return loss



# revision 9
# speedup vs baseline: 1.1775x; 1.1775x over previous
"""ChainCRF loss kernel for Trainium2 (8 NeuronCores, data-parallel over batch).

Math: the CRF forward recurrence
    part_t[j] = em[t, j] + logsumexp_i(part_{t-1}[i] + trans[i, j])
is computed in exp space:  V_t = E_t * (ET^T @ V_{t-1}),  E = exp(em - 8*ln2),
ET = exp(trans).  The per-step 2^-8 rescale keeps values in range; the absorbed
scale count is restored on the host.

Each of the 4 sequences per core is split into G=64 time-chunks of length 4,
processed as two half-phases (g<32 sources only even bt-tiles, so phase A
starts while the odd-tile gathers still run). All (batch, chunk) columns of a
half advance together through R=8 rounds of one [128,128] bf16 matmul
(stationary exp(trans)) + one elementwise multiply.
Chunks g>=1 start K=4 rounds early from a uniform vector: the Perron
contraction of the positive chain matrices makes the state direction converge,
so a chunk's state equals the true forward state up to a per-column scalar.
Those scalars are recovered on the host by matching each chunk's log-state at
its boundary time (snapshot after round K-1) against the previous chunk's
final state, averaging over the 128 labels.

tgt_energy = sum_t trans[prev_t, tgt_t] + em[t, tgt_t] is computed on device:
G2 = OHpT.T @ trans (OHpT is the host-encoded one-hot of prev, an index
relayout), X = em + G2, then a fused (iota == tgt) * X select-and-accumulate
per 128-row tile.

The embedding gather is 8 single-column SWDGE indirect DMAs (this image has no
extended-inst ucode, and multi-column offset APs degrade to one offset per
partition + linear continuation). The gathers are the ONLY Pool work: identity
/ iota / tgt_f come in via host-packed inputs so the gathers start as soon as
the token DMA lands and run back-to-back. Table is bf16 (halves transfer and
PE-transpose cost; precision is ample for the 2e-2 gate).

Every instruction is kept to at most ONE semaphore wait (this walrus build
rejects more): producers are grouped per engine, consumers ordered so earlier
waits cover later deps, small "observer" ops absorb extra cross-engine waits,
the chain writes a fresh state tile per round (same-engine WAW on DVE emits
waits), and the Tile end-of-kernel drain is split into single-wait drains.
"""

import numpy as np

# problem dims (hardcoded per contract)
B, L, VOCAB, C = 32, 256, 50000, 128
NCORES = 8
BPC = B // NCORES      # 4 sequences per core
G = 64                 # chunks per sequence
CL = L // G            # 4 steps per chunk
K = 4                  # burn-in rounds (K=CL makes each chunk's burn-in
                       # window coincide with the previous chunk's real window,
                       # so matching residuals cancel)
R = K + CL             # 8 rounds
F = BPC * G            # 256 chain columns per core
NT = (BPC * L) // 128  # 8 gather tiles of 128 rows per core
LN2 = 0.6931471805599453
SBITS = 8              # per-step rescale = 2^-SBITS
OUT_W = 2 * F + NT     # out: [128, logV(F) | logSnap(F) | tgtsum(8)]
# transx2 layout (f32): [trans(C) | trans_row127(1) | tgt_f(NT) | iota_row(C) | bias(1)]
TXW = C + 1 + NT + C + 1
# ohpx2 layout (bf16): [ohp(NT*128) | transb(C) | ident(C)]
OHW = NT * 128 + C + C


def _make_tc_class():
    import concourse.tile as tile
    from concourse.vector_clock import ScopedClock, VectorClock

    class SingleWaitTC(tile.TileContext):
        """TileContext whose end-of-kernel drain is split into single-wait
        sync-engine drains (this walrus rejects >1 wait per instruction)."""

        def _drain_and_barrier(self, tick_clock, wait_clock):
            nc = self.nc
            gc = tick_clock.global_clock
            n = len(gc)
            for p in range(n):
                t = gc[p]
                if t <= 0:
                    continue
                vec = [0] * n
                vec[p] = t
                nop = nc.sync.drain()
                wait_clock.add_sem_waits(
                    nop.ins, ScopedClock({None: VectorClock(vec)}))
            # per-proc drains above already waited on everything (including
            # the output DMA queues), so outputs are in DRAM; skip the EVSEM
            # butterfly barrier (~5-7us) and sem clears entirely — each
            # kernel() call loads a fresh NEFF, so semaphores start from
            # their load-time values
            nc.sync.drain()
            popped = nc._tile_sem_poison_stack.pop()
            assert popped is self._sem_poison

    return SingleWaitTC


def _build():
    import concourse.bass as bass
    import concourse.tile as tile
    from concourse import mybir

    f32 = mybir.dt.float32
    bf16 = mybir.dt.bfloat16
    i32 = mybir.dt.int32
    Alu = mybir.AluOpType
    Act = mybir.ActivationFunctionType

    nc = bass.Bass("TRN2", debug=False)

    table_d = nc.dram_tensor("tableb", [VOCAB, C], bf16,
                             kind="ExternalInput").ap()
    tok_d = nc.dram_tensor("tok", [128, NT], i32, kind="ExternalInput").ap()
    transx_d = nc.dram_tensor("transx", [128, TXW], f32,
                              kind="ExternalInput").ap()
    ohpx_d = nc.dram_tensor("ohpx", [128, OHW], bf16,
                            kind="ExternalInput").ap()
    out_d = nc.dram_tensor("out", [128, OUT_W], f32, kind="ExternalOutput").ap()

    def mkap(t_ap, offset, dims):
        # dims: list of [stride, count] free dims; partition dim prepended
        return bass.AP(t_ap.tensor, offset, [t_ap.ap[0]] + dims)

    TC = _make_tc_class()
    with TC(nc) as tc:
        with (
            tc.tile_pool(name="sb", bufs=1) as sb,
            tc.tile_pool(name="ps", bufs=1, space="PSUM") as psp,
        ):
            def st(shape, dt, nm):
                return sb.tile(shape, dt, name=nm, tag=nm)

            def pt(shape, dt, nm):
                return psp.tile(shape, dt, name=nm, tag=nm)

            # ---- input DMAs (tokens first and smallest) ----
            tok_sb = st([128, NT], i32, "tok_sb")
            nc.sync.dma_start(tok_sb[:], tok_d)
            transx_sb = st([128, TXW], f32, "transx_sb")
            nc.scalar.dma_start(transx_sb[:], transx_d)
            trans_sb = transx_sb[:, 0:C]
            tr127_sb = transx_sb[:, C:C + 1]
            tgt_f = transx_sb[:, C + 1:C + 1 + NT]
            iota_row = transx_sb[:, C + 1 + NT:C + 1 + NT + C]
            bias_t = transx_sb[:, TXW - 1:TXW]
            ohpx_sb = st([128, OHW], bf16, "ohpx_sb")
            nc.sync.dma_start(ohpx_sb[:], ohpx_d)
            ohp_sb = ohpx_sb[:, 0:NT * 128]
            transb_sb = ohpx_sb[:, NT * 128:NT * 128 + C]
            ident = ohpx_sb[:, NT * 128 + C:OHW]

            # ---- gpsimd: ONLY the 8 gathers (even bt-tiles first: they
            # feed chunks g<32, so phase A runs while odd gathers continue).
            # Nothing else on Pool — every prep constant is host-packed ----
            em_sb = st([128, NT * 128], bf16, "em_sb")
            GORDER = [0, 2, 4, 6, 1, 3, 5, 7]
            for k in GORDER:
                nc.gpsimd.indirect_dma_start(
                    out=em_sb[:, k * 128:(k + 1) * 128],
                    out_offset=None,
                    in_=table_d,
                    in_offset=bass.IndirectOffsetOnAxis(
                        ap=tok_sb[:, k:k + 1], axis=0),
                )

            GH = G // 2          # chunks per half
            FH = BPC * GH        # chain columns per half; col = s*FH + b*GH + g
            ewinA = st([128, R * FH], bf16, "ewinA")
            ewinB = st([128, R * FH], bf16, "ewinB")
            ewa = ewinA[:]
            ewb = ewinB[:]

            # ---- PE: observer (identity self-transpose; absorbs the ohpx
            # DMA queue sem), then even-tile transposes (bf16) ----
            emT = pt([128, NT * 128], bf16, "emT")
            x_ps = pt([128, NT * 128], f32, "x_ps")
            obsps = pt([128, 128], bf16, "obsps")
            nc.tensor.transpose(out=obsps[:], in_=ident,
                                identity=ident)
            for k in [0, 2, 4, 6]:
                nc.tensor.transpose(
                    out=emT[:, k * 128:(k + 1) * 128],
                    in_=em_sb[:, k * 128:(k + 1) * 128],
                    identity=ident,
                )

            # ---- ACT: constants + A-half exps ----
            ET = st([C, C], bf16, "ET")
            nc.scalar.activation(ET[:], trans_sb, Act.Exp)
            ET127 = st([C, 1], f32, "ET127")
            nc.scalar.activation(ET127[:], tr127_sb, Act.Exp)
            VA = [st([128, FH], bf16, f"VA{s}") for s in range(R + 1)]
            VB = [st([128, FH], bf16, f"VB{s}") for s in range(R + 1)]
            one = nc.const_aps.aps[(f32, 1.0)]
            nc.scalar.activation(VA[0][:], one[:128].to_broadcast([128, FH]),
                                 Act.Copy)
            nc.scalar.activation(VB[0][:], one[:128].to_broadcast([128, FH]),
                                 Act.Copy)
            # chunk-0 burn-in window = constant 2^-SBITS (was a Pool memset)
            nc.scalar.activation(
                mkap(ewa, 0, [[GH, BPC], [FH, K]]),
                one[:128].to_broadcast([128, BPC * K]),
                Act.Copy, scale=2.0 ** -SBITS)

            emt = emT[:]
            BIAS = bias_t
            # half A: g in [1, GH)  <- src t = CL*(g-1)+s in [0, 128)
            # (emitted before the odd transposes so the program-order RAW dep
            # set is the even transposes only)
            for b in range(BPC):
                nc.scalar.activation(
                    mkap(ewa, b * GH + 1, [[FH, R], [1, GH - 1]]),
                    mkap(emt, b * L, [[1, R], [CL, GH - 1]]),
                    Act.Exp, bias=BIAS)
            # chunk0 real steps: s in [K+1, R) <- t = s-K in [1, CL)
            nc.scalar.activation(
                mkap(ewa, (K + 1) * FH, [[GH, BPC], [FH, CL - 1]]),
                mkap(emt, 1, [[L, BPC], [1, CL - 1]]),
                Act.Exp, bias=BIAS)
            # E0 at chunk0 col s=K (re-init source; last A-feeding exp)
            nc.scalar.activation(
                mkap(ewa, K * FH, [[GH, BPC]]),
                mkap(emt, 0, [[L, BPC]]),
                Act.Exp, bias=BIAS)

            # PE observer of the A-exps (E0 was the last one), so the odd
            # transposes' WAR on emT doesn't add a second wait
            nc.tensor.matmul(out=x_ps[0:1, 0:1],
                             lhsT=ewinA[:, K * FH:K * FH + 1],
                             rhs=ewinA[:, K * FH:K * FH + 1],
                             start=True, stop=True)
            for k in [1, 3, 5, 7]:
                nc.tensor.transpose(
                    out=emT[:, k * 128:(k + 1) * 128],
                    in_=em_sb[:, k * 128:(k + 1) * 128],
                    identity=ident,
                )
            # half B: g in [GH, G)  <- src t = CL*(g-1)+s in [124, 256)
            # ACT observer first: absorbs the ACT-self tick the first B-exp
            # would otherwise carry as a second wait
            scrb = st([128, 1], f32, "scrb")
            obs_act = nc.scalar.activation(
                scrb[:1, 0:1], ewinA[0:1, K * FH:K * FH + 1], Act.Copy)
            first_bexp = None
            for b in range(BPC):
                h = nc.scalar.activation(
                    mkap(ewb, b * GH, [[FH, R], [1, GH]]),
                    mkap(emt, b * L + CL * (GH - 1), [[1, R], [CL, GH]]),
                    Act.Exp, bias=BIAS)
                if first_bexp is None:
                    first_bexp = h
                    tile.add_dep_helper(h.ins, obs_act.ins, sync=False,
                                        reason="order ACT obs before B exps")

            # ---- output staging: [logVA |snapA |logVB |snapB] ----
            outsb = st([128, 2 * F], f32, "outsb")
            tgtout = st([128, NT], f32, "tgtout")
            scr = st([128, 4], f32, "scr")

            # ---- the chains (high priority; B can interleave into A) ----
            psA = pt([128, FH], f32, "psA")
            psB = pt([128, FH], f32, "psB")
            with tc.high_priority():
                # DVE observers for chain A (must share the chain's priority
                # or the scheduler orders the chain TTs before them)
                nc.vector.tensor_copy(scr[:1, 0:1], ewinA[:1, 0:1])
                obs_e = nc.vector.tensor_copy(scr[:1, 1:2],
                                              ewinA[:1, K * FH:K * FH + 1])
                tt0 = None
                for s in range(R):
                    nc.tensor.matmul(out=psA[:], lhsT=ET[:], rhs=VA[s][:],
                                     start=True, stop=True)
                    h = nc.vector.tensor_tensor(
                        out=VA[s + 1][:], in0=psA[:],
                        in1=ewinA[:, s * FH:(s + 1) * FH], op=Alu.mult)
                    if tt0 is None:
                        tt0 = h
                        tile.add_dep_helper(h.ins, obs_e.ins, sync=False,
                                            reason="order DVE obs before TTs")
                    if s == K - 1:
                        nc.scalar.activation(outsb[:, F:F + FH], VA[K][:],
                                             Act.Ln)
                    if s == K:
                        # re-init chunk-0 columns (b*GH) from true part0
                        nc.vector.tensor_scalar_mul(
                            mkap(VA[K + 1][:], 0, [[GH, BPC]]),
                            mkap(ewa, K * FH, [[GH, BPC]]),
                            ET127[:],
                        )
                nc.scalar.activation(outsb[:, 0:FH], VA[R][:], Act.Ln)
                # observer: absorb the B-half exps' ACT tick before B's TTs
                nc.vector.tensor_copy(
                    scr[:1, 2:3], ewinB[:1, 3 * GH:3 * GH + 1])
                for s in range(R):
                    nc.tensor.matmul(out=psB[:], lhsT=ET[:], rhs=VB[s][:],
                                     start=True, stop=True)
                    nc.vector.tensor_tensor(
                        out=VB[s + 1][:], in0=psB[:],
                        in1=ewinB[:, s * FH:(s + 1) * FH], op=Alu.mult)
                    if s == K - 1:
                        nc.scalar.activation(outsb[:, F + FH:2 * F], VB[K][:],
                                             Act.Ln)
                nc.scalar.activation(outsb[:, FH:F], VB[R][:], Act.Ln)

            # A block out as soon as chain A is done (overlaps chain B)
            nc.sync.dma_start(out_d[:, 0:F], outsb[:, 0:F])

            # ---- tgt energy ----
            # G2 matmuls; a dummy matmul reading only ohp_sb absorbs its DMA
            # wait so each G2 matmul carries at most one (transb's queue)
            nc.tensor.matmul(out=x_ps[:, 0:128], lhsT=ohp_sb[:, 0:128],
                             rhs=ohp_sb[:, 0:128], start=True, stop=True)
            for k in range(NT):
                nc.tensor.matmul(
                    out=x_ps[:, k * 128:(k + 1) * 128],
                    lhsT=ohp_sb[:, k * 128:(k + 1) * 128],
                    rhs=transb_sb,
                    start=True, stop=True,
                )
            # DVE observers: per-queue gather sems + ACT tick (tgt_f/iota_row
            # arrive on the ACT HWDGE queue, covered via the ET exp;
            # emitted after the chains so they don't stall the chain TTs)
            scr8 = st([128, NT], f32, "scr8")
            for k in range(NT):
                nc.vector.tensor_copy(scr8[:1, k:k + 1],
                                      em_sb[:1, k * 128:k * 128 + 1])
            nc.vector.tensor_copy(scr[:1, 3:4], tgt_f[:1, 0:1])
            xs = st([128, NT * 128], f32, "xs")
            sel = st([128, NT * 128], f32, "sel")
            for k in range(NT):
                sl = slice(k * 128, (k + 1) * 128)
                nc.vector.tensor_tensor(out=xs[:, sl], in0=em_sb[:, sl],
                                        in1=x_ps[:, sl], op=Alu.add)
            for k in range(NT):
                sl = slice(k * 128, (k + 1) * 128)
                nc.vector.scalar_tensor_tensor(
                    out=sel[:, sl],
                    in0=iota_row,
                    scalar=tgt_f[:, k:k + 1],
                    in1=xs[:, sl],
                    op0=Alu.is_equal,
                    op1=Alu.mult,
                    accum_out=tgtout[:, k:k + 1],
                )

            nc.sync.dma_start(out_d[:, F:2 * F], outsb[:, F:2 * F])
            nc.sync.dma_start(out_d[:, 2 * F:OUT_W], tgtout[:])

    return nc


def _host_prep(tokens, target):
    """Per-core input maps. Index tensors are laid out column-major per
    128-row tile: arr[p, k] = flat[k*128 + p], flat index bt = b*L + t.
    ohp is the one-hot relayout of prev: ohp[i, bt] = (prev[bt] == i)."""
    import ml_dtypes
    bf16 = ml_dtypes.bfloat16
    tokens = np.ascontiguousarray(tokens, dtype=np.int32)
    target = np.ascontiguousarray(target, dtype=np.int32)
    prev = np.concatenate(
        [np.full((B, 1), C - 1, np.int32), target[:, :-1]], axis=1)
    iota = np.arange(C, dtype=np.int32)

    def cols(a):  # [BPC, L] -> [128, NT]
        return a.reshape(-1).reshape(NT, 128).T

    maps = []
    for c in range(NCORES):
        bs = slice(c * BPC, (c + 1) * BPC)
        pv = prev[bs].reshape(-1)  # [1024], bt order
        ohp = (pv[None, :] == iota[:, None]).astype(bf16)  # [128, 1024]
        maps.append({
            "tok": np.ascontiguousarray(cols(tokens[bs])),
            "_tgtf": cols(target[bs]).astype(np.float32),
            "_ohp": ohp,
        })
    return maps


def _combine(outs):
    """Stitch chunk states into per-batch loss. outs: list of [128, OUT_W]."""
    loss = np.empty(B, np.float64)
    sc = SBITS * LN2
    endcnt = np.full(G, R, np.float64)
    endcnt[0] = CL
    for c in range(NCORES):
        o = outs[c].astype(np.float64)
        FH = F // 2
        GH = G // 2
        lv = np.concatenate([o[:, 0:FH].reshape(C, BPC, GH),
                             o[:, FH:F].reshape(C, BPC, GH)], axis=2)
        ls = np.concatenate([o[:, F:F + FH].reshape(C, BPC, GH),
                             o[:, F + FH:2 * F].reshape(C, BPC, GH)], axis=2)
        tg = o[:, 2 * F:2 * F + NT]
        for bl in range(BPC):
            e = 0.0
            for g in range(1, G):
                d = (ls[:, bl, g] + K * sc) - (lv[:, bl, g - 1] + endcnt[g - 1] * sc)
                e += d.mean()
            part = lv[:, bl, G - 1] + endcnt[G - 1] * sc - e
            m = part.max()
            logz = np.log(np.exp(part - m).sum()) + m
            # tiles 2*bl, 2*bl+1 hold this sequence's bt rows
            tgt_e = tg[:, 2 * bl].sum() + tg[:, 2 * bl + 1].sum()
            loss[c * BPC + bl] = logz - tgt_e
    return loss.astype(np.float32)


def _run(inputs, trace=False):
    from concourse import bass_utils
    import ml_dtypes
    bf16 = ml_dtypes.bfloat16

    tokens = np.asarray(inputs["tokens"])
    target = np.asarray(inputs["target"])
    table = np.ascontiguousarray(np.asarray(inputs["state_table"], np.float32))
    trans = np.ascontiguousarray(np.asarray(inputs["trans_matrix"], np.float32))

    nc = _build()
    maps = _host_prep(tokens, target)
    tableb = np.ascontiguousarray(table.astype(bf16))
    transb = trans.astype(bf16)
    identb = np.eye(C, dtype=bf16)
    iota_row = np.broadcast_to(np.arange(C, dtype=np.float32), (128, C))
    for m in maps:
        ohp = m.pop("_ohp")
        tgtf = m.pop("_tgtf")
        m["ohpx"] = np.ascontiguousarray(
            np.concatenate([ohp, transb, identb], axis=1))
        m["transx"] = np.ascontiguousarray(np.concatenate(
            [trans, trans[C - 1:C, :].T, tgtf, iota_row,
             np.full((128, 1), -float(SBITS) * LN2, np.float32)], axis=1))
        m["tableb"] = tableb

    res = bass_utils.run_bass_kernel_spmd(
        nc, maps, core_ids=list(range(NCORES)), trace=trace)
    loss = _combine([r["out"] for r in res.results])
    return loss, res


def kernel(**inputs):
    loss, _ = _run(inputs, trace=False)
    return loss


# revision 10
# speedup vs baseline: 1.2232x; 1.0388x over previous
"""ChainCRF loss kernel for Trainium2 (8 NeuronCores, data-parallel over batch).

Math: the CRF forward recurrence
    part_t[j] = em[t, j] + logsumexp_i(part_{t-1}[i] + trans[i, j])
is computed in exp space:  V_t = E_t * (ET^T @ V_{t-1}),  E = exp(em - 8*ln2),
ET = exp(trans).  The per-step 2^-8 rescale keeps values in range; the absorbed
scale count is restored on the host.

Each of the 4 sequences per core is split into G=64 time-chunks of length 4,
processed as two half-phases (g<32 sources only even bt-tiles, so phase A
starts while the odd-tile gathers still run). All (batch, chunk) columns of a
half advance together through R=8 rounds of one [128,128] bf16 matmul
(stationary exp(trans)) + one elementwise multiply.
Chunks g>=1 start K=4 rounds early from a uniform vector: the Perron
contraction of the positive chain matrices makes the state direction converge,
so a chunk's state equals the true forward state up to a per-column scalar.
Those scalars are recovered on the host by matching each chunk's log-state at
its boundary time (snapshot after round K-1) against the previous chunk's
final state, averaging over the 128 labels.

tgt_energy = sum_t trans[prev_t, tgt_t] + em[t, tgt_t] is computed on device:
G2 = OHpT.T @ trans (OHpT is the host-encoded one-hot of prev, an index
relayout), X = em + G2, then a fused (iota == tgt) * X select-and-accumulate
per 128-row tile.

The embedding gather is 8 single-column SWDGE indirect DMAs (this image has no
extended-inst ucode, and multi-column offset APs degrade to one offset per
partition + linear continuation). The gathers are the ONLY Pool work: identity
/ iota / tgt_f come in via host-packed inputs so the gathers start as soon as
the token DMA lands and run back-to-back. Table is bf16 (halves transfer and
PE-transpose cost; precision is ample for the 2e-2 gate).

Every instruction is kept to at most ONE semaphore wait (this walrus build
rejects more): producers are grouped per engine, consumers ordered so earlier
waits cover later deps, small "observer" ops absorb extra cross-engine waits,
the chain writes a fresh state tile per round (same-engine WAW on DVE emits
waits), and the Tile end-of-kernel drain is split into single-wait drains.
"""

import numpy as np

# problem dims (hardcoded per contract)
B, L, VOCAB, C = 32, 256, 50000, 128
NCORES = 8
BPC = B // NCORES      # 4 sequences per core
G = 64                 # chunks per sequence
CL = L // G            # 4 steps per chunk
K = 4                  # burn-in rounds (K=CL makes each chunk's burn-in
                       # window coincide with the previous chunk's real window,
                       # so matching residuals cancel)
R = K + CL             # 8 rounds
F = BPC * G            # 256 chain columns per core
NT = (BPC * L) // 128  # 8 gather tiles of 128 rows per core
LN2 = 0.6931471805599453
SBITS = 8              # per-step rescale = 2^-SBITS
OUT_W = 2 * F + NT     # out: [128, logV(F) | logSnap(F) | tgtsum(8)]
# transx2 layout (f32): [trans(C) | trans_row127(1) | tgt_f(NT) | iota_row(C) | bias(1)]
TXW = C + 1 + NT + C + 1
# ohpx2 layout (bf16): [ohp(NT*128) | transb(C) | ident(C)]
OHW = NT * 128 + C + C


def _make_tc_class():
    import concourse.tile as tile
    from concourse.vector_clock import ScopedClock, VectorClock

    class SingleWaitTC(tile.TileContext):
        """TileContext whose end-of-kernel drain is split into single-wait
        sync-engine drains (this walrus rejects >1 wait per instruction)."""

        def _drain_and_barrier(self, tick_clock, wait_clock):
            nc = self.nc
            gc = tick_clock.global_clock
            n = len(gc)
            for p in range(n):
                t = gc[p]
                if t <= 0:
                    continue
                vec = [0] * n
                vec[p] = t
                nop = nc.sync.drain()
                wait_clock.add_sem_waits(
                    nop.ins, ScopedClock({None: VectorClock(vec)}))
            # per-proc drains above already waited on everything (including
            # the output DMA queues), so outputs are in DRAM; skip the EVSEM
            # butterfly barrier (~5-7us) and sem clears entirely — each
            # kernel() call loads a fresh NEFF, so semaphores start from
            # their load-time values
            nc.sync.drain()
            popped = nc._tile_sem_poison_stack.pop()
            assert popped is self._sem_poison

    return SingleWaitTC


def _build():
    import concourse.bass as bass
    import concourse.tile as tile
    from concourse import mybir

    f32 = mybir.dt.float32
    bf16 = mybir.dt.bfloat16
    i32 = mybir.dt.int32
    Alu = mybir.AluOpType
    Act = mybir.ActivationFunctionType

    nc = bass.Bass("TRN2", debug=False)

    table_d = nc.dram_tensor("tableb", [VOCAB, C], bf16,
                             kind="ExternalInput").ap()
    tok_d = nc.dram_tensor("tok", [128, NT], i32, kind="ExternalInput").ap()
    transx_d = nc.dram_tensor("transx", [128, TXW], f32,
                              kind="ExternalInput").ap()
    ohpx_d = nc.dram_tensor("ohpx", [128, OHW], bf16,
                            kind="ExternalInput").ap()
    out_d = nc.dram_tensor("out", [128, OUT_W], f32, kind="ExternalOutput").ap()

    def mkap(t_ap, offset, dims):
        # dims: list of [stride, count] free dims; partition dim prepended
        return bass.AP(t_ap.tensor, offset, [t_ap.ap[0]] + dims)

    TC = _make_tc_class()
    with TC(nc) as tc:
        with (
            tc.tile_pool(name="sb", bufs=1) as sb,
            tc.tile_pool(name="ps", bufs=1, space="PSUM") as psp,
        ):
            def st(shape, dt, nm):
                return sb.tile(shape, dt, name=nm, tag=nm)

            def pt(shape, dt, nm):
                return psp.tile(shape, dt, name=nm, tag=nm)

            # ---- input DMAs (tokens first and smallest) ----
            tok_sb = st([128, NT], i32, "tok_sb")
            nc.sync.dma_start(tok_sb[:], tok_d)
            transx_sb = st([128, TXW], f32, "transx_sb")
            nc.scalar.dma_start(transx_sb[:], transx_d)
            trans_sb = transx_sb[:, 0:C]
            tr127_sb = transx_sb[:, C:C + 1]
            tgt_f = transx_sb[:, C + 1:C + 1 + NT]
            iota_row = transx_sb[:, C + 1 + NT:C + 1 + NT + C]
            bias_t = transx_sb[:, TXW - 1:TXW]
            ohpx_sb = st([128, OHW], bf16, "ohpx_sb")
            nc.sync.dma_start(ohpx_sb[:], ohpx_d)
            ohp_sb = ohpx_sb[:, 0:NT * 128]
            transb_sb = ohpx_sb[:, NT * 128:NT * 128 + C]
            ident = ohpx_sb[:, NT * 128 + C:OHW]

            # ---- gpsimd: ONLY the 8 gathers (even bt-tiles first: they
            # feed chunks g<32, so phase A runs while odd gathers continue).
            # Nothing else on Pool — every prep constant is host-packed ----
            em_sb = st([128, NT * 128], bf16, "em_sb")
            GORDER = [0, 2, 4, 6, 1, 3, 5, 7]
            for k in GORDER:
                nc.gpsimd.indirect_dma_start(
                    out=em_sb[:, k * 128:(k + 1) * 128],
                    out_offset=None,
                    in_=table_d,
                    in_offset=bass.IndirectOffsetOnAxis(
                        ap=tok_sb[:, k:k + 1], axis=0),
                )

            GH = G // 2          # chunks per half
            FH = BPC * GH        # chain columns per half; col = s*FH + b*GH + g
            ewinA = st([128, R * FH], bf16, "ewinA")
            ewinB = st([128, R * FH], bf16, "ewinB")
            ewa = ewinA[:]
            ewb = ewinB[:]

            # ---- PE: observer (identity self-transpose; absorbs the ohpx
            # DMA queue sem), then even-tile transposes (bf16) ----
            emT = pt([128, NT * 128], bf16, "emT")
            x_ps = pt([128, NT * 128], f32, "x_ps")
            obsps = pt([128, 128], bf16, "obsps")
            nc.tensor.transpose(out=obsps[:], in_=ident,
                                identity=ident)
            for k in [0, 2, 4, 6]:
                nc.tensor.transpose(
                    out=emT[:, k * 128:(k + 1) * 128],
                    in_=em_sb[:, k * 128:(k + 1) * 128],
                    identity=ident,
                )

            # ---- ACT: constants + A-half exps ----
            ET = st([C, C], bf16, "ET")
            nc.scalar.activation(ET[:], trans_sb, Act.Exp)
            ET127 = st([C, 1], f32, "ET127")
            nc.scalar.activation(ET127[:], tr127_sb, Act.Exp)
            VA = [st([128, FH], bf16, f"VA{s}") for s in range(R + 1)]
            VB = [st([128, FH], bf16, f"VB{s}") for s in range(R + 1)]
            one = nc.const_aps.aps[(f32, 1.0)]
            nc.scalar.activation(VA[0][:], one[:128].to_broadcast([128, FH]),
                                 Act.Copy)
            nc.scalar.activation(VB[0][:], one[:128].to_broadcast([128, FH]),
                                 Act.Copy)
            # chunk-0 burn-in window = constant 2^-SBITS (was a Pool memset)
            nc.scalar.activation(
                mkap(ewa, 0, [[GH, BPC], [FH, K]]),
                one[:128].to_broadcast([128, BPC * K]),
                Act.Copy, scale=2.0 ** -SBITS)

            emt = emT[:]
            BIAS = bias_t
            # half A: g in [1, GH)  <- src t = CL*(g-1)+s in [0, 128)
            # (emitted before the odd transposes so the program-order RAW dep
            # set is the even transposes only)
            for b in range(BPC):
                nc.scalar.activation(
                    mkap(ewa, b * GH + 1, [[FH, R], [1, GH - 1]]),
                    mkap(emt, b * L, [[1, R], [CL, GH - 1]]),
                    Act.Exp, bias=BIAS)
            # chunk0 real steps: s in [K+1, R) <- t = s-K in [1, CL)
            nc.scalar.activation(
                mkap(ewa, (K + 1) * FH, [[GH, BPC], [FH, CL - 1]]),
                mkap(emt, 1, [[L, BPC], [1, CL - 1]]),
                Act.Exp, bias=BIAS)
            # E0 at chunk0 col s=K (re-init source; last A-feeding exp)
            nc.scalar.activation(
                mkap(ewa, K * FH, [[GH, BPC]]),
                mkap(emt, 0, [[L, BPC]]),
                Act.Exp, bias=BIAS)

            # PE observer of the A-exps (E0 was the last one), so the odd
            # transposes' WAR on emT doesn't add a second wait
            nc.tensor.matmul(out=x_ps[0:1, 0:1],
                             lhsT=ewinA[:, K * FH:K * FH + 1],
                             rhs=ewinA[:, K * FH:K * FH + 1],
                             start=True, stop=True)
            for k in [1, 3, 5, 7]:
                nc.tensor.transpose(
                    out=emT[:, k * 128:(k + 1) * 128],
                    in_=em_sb[:, k * 128:(k + 1) * 128],
                    identity=ident,
                )
            # half B: g in [GH, G)  <- src t = CL*(g-1)+s in [124, 256)
            # ACT observer first: absorbs the ACT-self tick the first B-exp
            # would otherwise carry as a second wait
            scrb = st([128, 1], f32, "scrb")
            obs_act = nc.scalar.activation(
                scrb[:1, 0:1], ewinA[0:1, K * FH:K * FH + 1], Act.Copy)
            first_bexp = None
            for b in range(BPC):
                h = nc.scalar.activation(
                    mkap(ewb, b * GH, [[FH, R], [1, GH]]),
                    mkap(emt, b * L + CL * (GH - 1), [[1, R], [CL, GH]]),
                    Act.Exp, bias=BIAS)
                if first_bexp is None:
                    first_bexp = h
                    tile.add_dep_helper(h.ins, obs_act.ins, sync=False,
                                        reason="order ACT obs before B exps")

            # ---- output staging: [logVA |snapA |logVB |snapB] ----
            outsb = st([128, 2 * F], f32, "outsb")
            tgtout = st([128, NT], f32, "tgtout")
            scr = st([128, 4], f32, "scr")

            # ---- the chains (high priority; B can interleave into A) ----
            psA = pt([128, FH], f32, "psA")
            psB = pt([128, FH], f32, "psB")
            with tc.high_priority():
                # DVE observers for chain A (must share the chain's priority
                # or the scheduler orders the chain TTs before them)
                nc.vector.tensor_copy(scr[:1, 0:1], ewinA[:1, 0:1])
                obs_e = nc.vector.tensor_copy(scr[:1, 1:2],
                                              ewinA[:1, K * FH:K * FH + 1])
                tt0 = None
                for s in range(R):
                    nc.tensor.matmul(out=psA[:], lhsT=ET[:], rhs=VA[s][:],
                                     start=True, stop=True)
                    h = nc.vector.tensor_tensor(
                        out=VA[s + 1][:], in0=psA[:],
                        in1=ewinA[:, s * FH:(s + 1) * FH], op=Alu.mult)
                    if tt0 is None:
                        tt0 = h
                        tile.add_dep_helper(h.ins, obs_e.ins, sync=False,
                                            reason="order DVE obs before TTs")
                    if s == K - 1:
                        nc.scalar.activation(outsb[:, F:F + FH], VA[K][:],
                                             Act.Ln)
                    if s == K:
                        # re-init chunk-0 columns (b*GH) from true part0
                        nc.vector.tensor_scalar_mul(
                            mkap(VA[K + 1][:], 0, [[GH, BPC]]),
                            mkap(ewa, K * FH, [[GH, BPC]]),
                            ET127[:],
                        )
                nc.scalar.activation(outsb[:, 0:FH], VA[R][:], Act.Ln)
                # observer: absorb the B-half exps' ACT tick before B's TTs
                nc.vector.tensor_copy(
                    scr[:1, 2:3], ewinB[:1, 3 * GH:3 * GH + 1])
                for s in range(R):
                    nc.tensor.matmul(out=psB[:], lhsT=ET[:], rhs=VB[s][:],
                                     start=True, stop=True)
                    nc.vector.tensor_tensor(
                        out=VB[s + 1][:], in0=psB[:],
                        in1=ewinB[:, s * FH:(s + 1) * FH], op=Alu.mult)
                    if s == K - 1:
                        nc.scalar.activation(outsb[:, F + FH:2 * F], VB[K][:],
                                             Act.Ln)
                nc.scalar.activation(outsb[:, FH:F], VB[R][:], Act.Ln)

            # A block out as soon as chain A is done (overlaps chain B)
            nc.sync.dma_start(out_d[:, 0:F], outsb[:, 0:F])

            # ---- tgt energy ----
            # G2 matmuls; a dummy matmul reading only ohp_sb absorbs its DMA
            # wait so each G2 matmul carries at most one (transb's queue)
            nc.tensor.matmul(out=x_ps[:, 0:128], lhsT=ohp_sb[:, 0:128],
                             rhs=ohp_sb[:, 0:128], start=True, stop=True)
            for k in range(NT):
                nc.tensor.matmul(
                    out=x_ps[:, k * 128:(k + 1) * 128],
                    lhsT=ohp_sb[:, k * 128:(k + 1) * 128],
                    rhs=transb_sb,
                    start=True, stop=True,
                )
            # DVE observers: per-queue gather sems + ACT tick (tgt_f/iota_row
            # arrive on the ACT HWDGE queue, covered via the ET exp;
            # emitted after the chains so they don't stall the chain TTs)
            scr8 = st([128, NT], f32, "scr8")
            for k in range(NT):
                nc.vector.tensor_copy(scr8[:1, k:k + 1],
                                      em_sb[:1, k * 128:k * 128 + 1])
            nc.vector.tensor_copy(scr[:1, 3:4], tgt_f[:1, 0:1])
            xs = st([128, NT * 128], f32, "xs")
            sel = st([128, NT * 128], f32, "sel")
            # pairwise xs/sel so the scheduler can slot each sel into a
            # chain-round gap instead of serializing all sels after the
            # last xs
            for k in range(NT):
                sl = slice(k * 128, (k + 1) * 128)
                nc.vector.tensor_tensor(out=xs[:, sl], in0=em_sb[:, sl],
                                        in1=x_ps[:, sl], op=Alu.add)
                nc.vector.scalar_tensor_tensor(
                    out=sel[:, sl],
                    in0=iota_row,
                    scalar=tgt_f[:, k:k + 1],
                    in1=xs[:, sl],
                    op0=Alu.is_equal,
                    op1=Alu.mult,
                    accum_out=tgtout[:, k:k + 1],
                )

            nc.sync.dma_start(out_d[:, F:2 * F], outsb[:, F:2 * F])
            nc.sync.dma_start(out_d[:, 2 * F:OUT_W], tgtout[:])

    return nc


def _host_prep(tokens, target):
    """Per-core input maps. Index tensors are laid out column-major per
    128-row tile: arr[p, k] = flat[k*128 + p], flat index bt = b*L + t.
    ohp is the one-hot relayout of prev: ohp[i, bt] = (prev[bt] == i)."""
    import ml_dtypes
    bf16 = ml_dtypes.bfloat16
    tokens = np.ascontiguousarray(tokens, dtype=np.int32)
    target = np.ascontiguousarray(target, dtype=np.int32)
    prev = np.concatenate(
        [np.full((B, 1), C - 1, np.int32), target[:, :-1]], axis=1)
    iota = np.arange(C, dtype=np.int32)

    def cols(a):  # [BPC, L] -> [128, NT]
        return a.reshape(-1).reshape(NT, 128).T

    maps = []
    for c in range(NCORES):
        bs = slice(c * BPC, (c + 1) * BPC)
        pv = prev[bs].reshape(-1)  # [1024], bt order
        ohp = (pv[None, :] == iota[:, None]).astype(bf16)  # [128, 1024]
        maps.append({
            "tok": np.ascontiguousarray(cols(tokens[bs])),
            "_tgtf": cols(target[bs]).astype(np.float32),
            "_ohp": ohp,
        })
    return maps


def _combine(outs):
    """Stitch chunk states into per-batch loss. outs: list of [128, OUT_W]."""
    loss = np.empty(B, np.float64)
    sc = SBITS * LN2
    endcnt = np.full(G, R, np.float64)
    endcnt[0] = CL
    for c in range(NCORES):
        o = outs[c].astype(np.float64)
        FH = F // 2
        GH = G // 2
        lv = np.concatenate([o[:, 0:FH].reshape(C, BPC, GH),
                             o[:, FH:F].reshape(C, BPC, GH)], axis=2)
        ls = np.concatenate([o[:, F:F + FH].reshape(C, BPC, GH),
                             o[:, F + FH:2 * F].reshape(C, BPC, GH)], axis=2)
        tg = o[:, 2 * F:2 * F + NT]
        for bl in range(BPC):
            e = 0.0
            for g in range(1, G):
                d = (ls[:, bl, g] + K * sc) - (lv[:, bl, g - 1] + endcnt[g - 1] * sc)
                e += d.mean()
            part = lv[:, bl, G - 1] + endcnt[G - 1] * sc - e
            m = part.max()
            logz = np.log(np.exp(part - m).sum()) + m
            # tiles 2*bl, 2*bl+1 hold this sequence's bt rows
            tgt_e = tg[:, 2 * bl].sum() + tg[:, 2 * bl + 1].sum()
            loss[c * BPC + bl] = logz - tgt_e
    return loss.astype(np.float32)


def _run(inputs, trace=False):
    from concourse import bass_utils
    import ml_dtypes
    bf16 = ml_dtypes.bfloat16

    tokens = np.asarray(inputs["tokens"])
    target = np.asarray(inputs["target"])
    table = np.ascontiguousarray(np.asarray(inputs["state_table"], np.float32))
    trans = np.ascontiguousarray(np.asarray(inputs["trans_matrix"], np.float32))

    nc = _build()
    maps = _host_prep(tokens, target)
    tableb = np.ascontiguousarray(table.astype(bf16))
    transb = trans.astype(bf16)
    identb = np.eye(C, dtype=bf16)
    iota_row = np.broadcast_to(np.arange(C, dtype=np.float32), (128, C))
    for m in maps:
        ohp = m.pop("_ohp")
        tgtf = m.pop("_tgtf")
        m["ohpx"] = np.ascontiguousarray(
            np.concatenate([ohp, transb, identb], axis=1))
        m["transx"] = np.ascontiguousarray(np.concatenate(
            [trans, trans[C - 1:C, :].T, tgtf, iota_row,
             np.full((128, 1), -float(SBITS) * LN2, np.float32)], axis=1))
        m["tableb"] = tableb

    res = bass_utils.run_bass_kernel_spmd(
        nc, maps, core_ids=list(range(NCORES)), trace=trace)
    loss = _combine([r["out"] for r in res.results])
    return loss, res


def kernel(**inputs):
    loss, _ = _run(inputs, trace=False)
    return loss


# revision 14
# speedup vs baseline: 1.2242x; 1.0008x over previous
"""ChainCRF loss kernel for Trainium2 (8 NeuronCores, data-parallel over batch).

Math: the CRF forward recurrence
    part_t[j] = em[t, j] + logsumexp_i(part_{t-1}[i] + trans[i, j])
is computed in exp space:  V_t = E_t * (ET^T @ V_{t-1}),  E = exp(em - 8*ln2),
ET = exp(trans).  The per-step 2^-8 rescale keeps values in range; the absorbed
scale count is restored on the host.

Each of the 4 sequences per core is split into G=64 time-chunks of length 4,
processed as two half-phases (g<32 sources only even bt-tiles, so phase A
starts while the odd-tile gathers still run). All (batch, chunk) columns of a
half advance together through R=8 rounds of one [128,128] bf16 matmul
(stationary exp(trans)) + one elementwise multiply.
Chunks g>=1 start K=4 rounds early from a uniform vector: the Perron
contraction of the positive chain matrices makes the state direction converge,
so a chunk's state equals the true forward state up to a per-column scalar.
Those scalars are recovered on the host by matching each chunk's log-state at
its boundary time (snapshot after round K-1) against the previous chunk's
final state, averaging over the 128 labels.

tgt_energy = sum_t trans[prev_t, tgt_t] + em[t, tgt_t] is computed on device:
G2 = OHpT.T @ trans (OHpT is the host-encoded one-hot of prev, an index
relayout), X = em + G2, then a fused (iota == tgt) * X select-and-accumulate
per 128-row tile.

The embedding gather is 8 single-column SWDGE indirect DMAs (this image has no
extended-inst ucode, and multi-column offset APs degrade to one offset per
partition + linear continuation). The gathers are the ONLY Pool work: identity
/ iota / tgt_f come in via host-packed inputs so the gathers start as soon as
the token DMA lands and run back-to-back. Table is bf16 (halves transfer and
PE-transpose cost; precision is ample for the 2e-2 gate).

Every instruction is kept to at most ONE semaphore wait (this walrus build
rejects more): producers are grouped per engine, consumers ordered so earlier
waits cover later deps, small "observer" ops absorb extra cross-engine waits,
the chain writes a fresh state tile per round (same-engine WAW on DVE emits
waits), and the Tile end-of-kernel drain is split into single-wait drains.
"""

import numpy as np

# problem dims (hardcoded per contract)
B, L, VOCAB, C = 32, 256, 50000, 128
NCORES = 8
BPC = B // NCORES      # 4 sequences per core
G = 64                 # chunks per sequence
CL = L // G            # 4 steps per chunk
K = 4                  # burn-in rounds (K=CL makes each chunk's burn-in
                       # window coincide with the previous chunk's real window,
                       # so matching residuals cancel)
R = K + CL             # 8 rounds
F = BPC * G            # 256 chain columns per core
NT = (BPC * L) // 128  # 8 gather tiles of 128 rows per core
LN2 = 0.6931471805599453
SBITS = 8              # per-step rescale = 2^-SBITS
OUT_W = 2 * F + NT     # out: [128, logV(F) | logSnap(F) | tgtsum(8)]
# transx2 layout (f32): [trans(C) | trans_row127(1) | tgt_f(NT) | iota_row(C) | bias(1)]
TXW = C + 1 + NT + C + 1
# ohpx2 layout (bf16): [ohp(NT*128) | transb(C) | ident(C)]
OHW = NT * 128 + C + C


def _make_tc_class():
    import concourse.tile as tile
    from concourse.vector_clock import ScopedClock, VectorClock

    class SingleWaitTC(tile.TileContext):
        """TileContext whose end-of-kernel drain is split into single-wait
        sync-engine drains (this walrus rejects >1 wait per instruction)."""

        def _drain_and_barrier(self, tick_clock, wait_clock):
            nc = self.nc
            gc = tick_clock.global_clock
            n = len(gc)
            for p in range(n):
                t = gc[p]
                if t <= 0:
                    continue
                vec = [0] * n
                vec[p] = t
                nop = nc.sync.drain()
                wait_clock.add_sem_waits(
                    nop.ins, ScopedClock({None: VectorClock(vec)}))
            # per-proc drains above already waited on everything (including
            # the output DMA queues), so outputs are in DRAM; skip the EVSEM
            # butterfly barrier (~5-7us) and sem clears entirely — each
            # kernel() call loads a fresh NEFF, so semaphores start from
            # their load-time values
            nc.sync.drain()
            popped = nc._tile_sem_poison_stack.pop()
            assert popped is self._sem_poison

    return SingleWaitTC


def _build():
    import concourse.bass as bass
    import concourse.tile as tile
    from concourse import mybir

    f32 = mybir.dt.float32
    bf16 = mybir.dt.bfloat16
    i32 = mybir.dt.int32
    Alu = mybir.AluOpType
    Act = mybir.ActivationFunctionType

    nc = bass.Bass("TRN2", debug=False)

    table_d = nc.dram_tensor("tableb", [VOCAB, C], bf16,
                             kind="ExternalInput").ap()
    tok_d = nc.dram_tensor("tok", [128, NT], i32, kind="ExternalInput").ap()
    transx_d = nc.dram_tensor("transx", [128, TXW], f32,
                              kind="ExternalInput").ap()
    ohpx_d = nc.dram_tensor("ohpx", [128, OHW], bf16,
                            kind="ExternalInput").ap()
    out_d = nc.dram_tensor("out", [128, OUT_W], f32, kind="ExternalOutput").ap()

    def mkap(t_ap, offset, dims):
        # dims: list of [stride, count] free dims; partition dim prepended
        return bass.AP(t_ap.tensor, offset, [t_ap.ap[0]] + dims)

    TC = _make_tc_class()
    with TC(nc) as tc:
        with (
            tc.tile_pool(name="sb", bufs=1) as sb,
            tc.tile_pool(name="ps", bufs=1, space="PSUM") as psp,
        ):
            def st(shape, dt, nm):
                return sb.tile(shape, dt, name=nm, tag=nm)

            def pt(shape, dt, nm):
                return psp.tile(shape, dt, name=nm, tag=nm)

            # ---- input DMAs (tokens first and smallest) ----
            tok_sb = st([128, NT], i32, "tok_sb")
            nc.sync.dma_start(tok_sb[:], tok_d)
            transx_sb = st([128, TXW], f32, "transx_sb")
            nc.scalar.dma_start(transx_sb[:], transx_d)
            trans_sb = transx_sb[:, 0:C]
            tr127_sb = transx_sb[:, C:C + 1]
            tgt_f = transx_sb[:, C + 1:C + 1 + NT]
            iota_row = transx_sb[:, C + 1 + NT:C + 1 + NT + C]
            bias_t = transx_sb[:, TXW - 1:TXW]
            ohpx_sb = st([128, OHW], bf16, "ohpx_sb")
            nc.sync.dma_start(ohpx_sb[:], ohpx_d)
            ohp_sb = ohpx_sb[:, 0:NT * 128]
            transb_sb = ohpx_sb[:, NT * 128:NT * 128 + C]
            ident = ohpx_sb[:, NT * 128 + C:OHW]

            # ---- gpsimd: ONLY the 8 gathers (even bt-tiles first: they
            # feed chunks g<32, so phase A runs while odd gathers continue).
            # Nothing else on Pool — every prep constant is host-packed ----
            em_sb = st([128, NT * 128], bf16, "em_sb")
            GORDER = [0, 2, 4, 6, 1, 3, 5, 7]
            for k in GORDER:
                nc.gpsimd.indirect_dma_start(
                    out=em_sb[:, k * 128:(k + 1) * 128],
                    out_offset=None,
                    in_=table_d,
                    in_offset=bass.IndirectOffsetOnAxis(
                        ap=tok_sb[:, k:k + 1], axis=0),
                )

            GH = G // 2          # chunks per half
            FH = BPC * GH        # chain columns per half; col = s*FH + b*GH + g
            ewinA = st([128, R * FH], bf16, "ewinA")
            ewinB = st([128, R * FH], bf16, "ewinB")
            ewa = ewinA[:]
            ewb = ewinB[:]

            # ---- PE: observer (identity self-transpose; absorbs the ohpx
            # DMA queue sem), then even-tile transposes (bf16) ----
            emT = pt([128, NT * 128], bf16, "emT")
            x_ps = pt([128, NT * 128], f32, "x_ps")
            obsps = pt([128, 128], bf16, "obsps")
            nc.tensor.transpose(out=obsps[:], in_=ident,
                                identity=ident)
            for k in [0, 2, 4, 6]:
                nc.tensor.transpose(
                    out=emT[:, k * 128:(k + 1) * 128],
                    in_=em_sb[:, k * 128:(k + 1) * 128],
                    identity=ident,
                )

            # ---- ACT: constants + A-half exps ----
            ET = st([C, C], bf16, "ET")
            nc.scalar.activation(ET[:], trans_sb, Act.Exp)
            ET127 = st([C, 1], f32, "ET127")
            nc.scalar.activation(ET127[:], tr127_sb, Act.Exp)
            VA = [st([128, FH], bf16, f"VA{s}") for s in range(R + 1)]
            VB = [st([128, FH], bf16, f"VB{s}") for s in range(R + 1)]
            one = nc.const_aps.aps[(f32, 1.0)]
            nc.scalar.activation(VA[0][:], one[:128].to_broadcast([128, FH]),
                                 Act.Copy)
            nc.scalar.activation(VB[0][:], one[:128].to_broadcast([128, FH]),
                                 Act.Copy)
            # chunk-0 burn-in window = constant 2^-SBITS (was a Pool memset)
            nc.scalar.activation(
                mkap(ewa, 0, [[GH, BPC], [FH, K]]),
                one[:128].to_broadcast([128, BPC * K]),
                Act.Copy, scale=2.0 ** -SBITS)

            emt = emT[:]
            BIAS = bias_t
            # half A: g in [1, GH)  <- src t = CL*(g-1)+s in [0, 128)
            # (emitted before the odd transposes so the program-order RAW dep
            # set is the even transposes only)
            for b in range(BPC):
                nc.scalar.activation(
                    mkap(ewa, b * GH + 1, [[FH, R], [1, GH - 1]]),
                    mkap(emt, b * L, [[1, R], [CL, GH - 1]]),
                    Act.Exp, bias=BIAS)
            # chunk0 real steps: s in [K+1, R) <- t = s-K in [1, CL)
            nc.scalar.activation(
                mkap(ewa, (K + 1) * FH, [[GH, BPC], [FH, CL - 1]]),
                mkap(emt, 1, [[L, BPC], [1, CL - 1]]),
                Act.Exp, bias=BIAS)
            # E0 at chunk0 col s=K (re-init source; last A-feeding exp)
            nc.scalar.activation(
                mkap(ewa, K * FH, [[GH, BPC]]),
                mkap(emt, 0, [[L, BPC]]),
                Act.Exp, bias=BIAS)

            # PE observer of the A-exps (E0 was the last one), so the odd
            # transposes' WAR on emT doesn't add a second wait
            nc.tensor.matmul(out=x_ps[0:1, 0:1],
                             lhsT=ewinA[:, K * FH:K * FH + 1],
                             rhs=ewinA[:, K * FH:K * FH + 1],
                             start=True, stop=True)
            for k in [1, 3, 5, 7]:
                nc.tensor.transpose(
                    out=emT[:, k * 128:(k + 1) * 128],
                    in_=em_sb[:, k * 128:(k + 1) * 128],
                    identity=ident,
                )
            # half B: g in [GH, G)  <- src t = CL*(g-1)+s in [124, 256)
            # ACT observer first: absorbs the ACT-self tick the first B-exp
            # would otherwise carry as a second wait
            scrb = st([128, 1], f32, "scrb")
            obs_act = nc.scalar.activation(
                scrb[:1, 0:1], ewinA[0:1, K * FH:K * FH + 1], Act.Copy)
            first_bexp = None
            for b in range(BPC):
                h = nc.scalar.activation(
                    mkap(ewb, b * GH, [[FH, R], [1, GH]]),
                    mkap(emt, b * L + CL * (GH - 1), [[1, R], [CL, GH]]),
                    Act.Exp, bias=BIAS)
                if first_bexp is None:
                    first_bexp = h
                    tile.add_dep_helper(h.ins, obs_act.ins, sync=False,
                                        reason="order ACT obs before B exps")

            # ---- output staging: [logVA |snapA |logVB |snapB] ----
            outsb = st([128, 2 * F], f32, "outsb")
            tgtout = st([128, NT], f32, "tgtout")
            scr = st([128, 4], f32, "scr")

            # ---- the chains (high priority; B can interleave into A) ----
            psA = pt([128, FH], f32, "psA")
            psB = pt([128, FH], f32, "psB")
            with tc.high_priority():
                # DVE observers for chain A (must share the chain's priority
                # or the scheduler orders the chain TTs before them)
                nc.vector.tensor_copy(scr[:1, 0:1], ewinA[:1, 0:1])
                obs_e = nc.vector.tensor_copy(scr[:1, 1:2],
                                              ewinA[:1, K * FH:K * FH + 1])
                tt0 = None
                for s in range(R):
                    nc.tensor.matmul(out=psA[:], lhsT=ET[:], rhs=VA[s][:],
                                     start=True, stop=True)
                    h = nc.vector.tensor_tensor(
                        out=VA[s + 1][:], in0=psA[:],
                        in1=ewinA[:, s * FH:(s + 1) * FH], op=Alu.mult)
                    if tt0 is None:
                        tt0 = h
                        tile.add_dep_helper(h.ins, obs_e.ins, sync=False,
                                            reason="order DVE obs before TTs")
                    if s == K - 1:
                        nc.scalar.activation(outsb[:, F:F + FH], VA[K][:],
                                             Act.Ln)
                    if s == K:
                        # re-init chunk-0 columns (b*GH) from true part0
                        nc.vector.tensor_scalar_mul(
                            mkap(VA[K + 1][:], 0, [[GH, BPC]]),
                            mkap(ewa, K * FH, [[GH, BPC]]),
                            ET127[:],
                        )
                nc.scalar.activation(outsb[:, 0:FH], VA[R][:], Act.Ln)
                # observer: absorb the B-half exps' ACT tick before B's TTs
                nc.vector.tensor_copy(
                    scr[:1, 2:3], ewinB[:1, 3 * GH:3 * GH + 1])
                for s in range(R):
                    nc.tensor.matmul(out=psB[:], lhsT=ET[:], rhs=VB[s][:],
                                     start=True, stop=True)
                    nc.vector.tensor_tensor(
                        out=VB[s + 1][:], in0=psB[:],
                        in1=ewinB[:, s * FH:(s + 1) * FH], op=Alu.mult)
                    if s == K - 1:
                        nc.scalar.activation(outsb[:, F + FH:2 * F], VB[K][:],
                                             Act.Ln)
                nc.scalar.activation(outsb[:, FH:F], VB[R][:], Act.Ln)

            # A block out as soon as chain A is done (overlaps chain B)
            nc.sync.dma_start(out_d[:, 0:F], outsb[:, 0:F])

            # ---- tgt energy ----
            # G2 matmuls; a dummy matmul reading only ohp_sb absorbs its DMA
            # wait so each G2 matmul carries at most one (transb's queue)
            nc.tensor.matmul(out=x_ps[:, 0:128], lhsT=ohp_sb[:, 0:128],
                             rhs=ohp_sb[:, 0:128], start=True, stop=True)
            for k in range(NT):
                nc.tensor.matmul(
                    out=x_ps[:, k * 128:(k + 1) * 128],
                    lhsT=ohp_sb[:, k * 128:(k + 1) * 128],
                    rhs=transb_sb,
                    start=True, stop=True,
                )
            # DVE observers: per-queue gather sems + ACT tick (tgt_f/iota_row
            # arrive on the ACT HWDGE queue, covered via the ET exp;
            # emitted after the chains so they don't stall the chain TTs)
            scr8 = st([128, NT], f32, "scr8")
            for k in range(NT):
                nc.vector.tensor_copy(scr8[:1, k:k + 1],
                                      em_sb[:1, k * 128:k * 128 + 1])
            nc.vector.tensor_copy(scr[:1, 3:4], tgt_f[:1, 0:1])
            xs = st([128, NT * 128], f32, "xs")
            sel = st([128, NT * 128], f32, "sel")
            # pairwise xs/sel so the scheduler can slot each sel into a
            # chain-round gap instead of serializing all sels after the
            # last xs
            for k in range(NT):
                sl = slice(k * 128, (k + 1) * 128)
                nc.vector.tensor_tensor(out=xs[:, sl], in0=em_sb[:, sl],
                                        in1=x_ps[:, sl], op=Alu.add)
                nc.vector.scalar_tensor_tensor(
                    out=sel[:, sl],
                    in0=iota_row,
                    scalar=tgt_f[:, k:k + 1],
                    in1=xs[:, sl],
                    op0=Alu.is_equal,
                    op1=Alu.mult,
                    accum_out=tgtout[:, k:k + 1],
                )

            nc.sync.dma_start(out_d[:, F:2 * F], outsb[:, F:2 * F])
            nc.sync.dma_start(out_d[:, 2 * F:OUT_W], tgtout[:])

    return nc


def _host_prep(tokens, target):
    """Per-core input maps. Index tensors are laid out column-major per
    128-row tile: arr[p, k] = flat[k*128 + p], flat index bt = b*L + t.
    ohp is the one-hot relayout of prev: ohp[i, bt] = (prev[bt] == i)."""
    import ml_dtypes
    bf16 = ml_dtypes.bfloat16
    tokens = np.ascontiguousarray(tokens, dtype=np.int32)
    target = np.ascontiguousarray(target, dtype=np.int32)
    prev = np.concatenate(
        [np.full((B, 1), C - 1, np.int32), target[:, :-1]], axis=1)
    iota = np.arange(C, dtype=np.int32)

    def cols(a):  # [BPC, L] -> [128, NT]
        return a.reshape(-1).reshape(NT, 128).T

    maps = []
    for c in range(NCORES):
        bs = slice(c * BPC, (c + 1) * BPC)
        pv = prev[bs].reshape(-1)  # [1024], bt order
        ohp = (pv[None, :] == iota[:, None]).astype(bf16)  # [128, 1024]
        maps.append({
            "tok": np.ascontiguousarray(cols(tokens[bs])),
            "_tgtf": cols(target[bs]).astype(np.float32),
            "_ohp": ohp,
        })
    return maps


def _combine(outs):
    """Stitch chunk states into per-batch loss. outs: list of [128, OUT_W]."""
    loss = np.empty(B, np.float64)
    sc = SBITS * LN2
    endcnt = np.full(G, R, np.float64)
    endcnt[0] = CL
    for c in range(NCORES):
        o = outs[c].astype(np.float64)
        FH = F // 2
        GH = G // 2
        lv = np.concatenate([o[:, 0:FH].reshape(C, BPC, GH),
                             o[:, FH:F].reshape(C, BPC, GH)], axis=2)
        ls = np.concatenate([o[:, F:F + FH].reshape(C, BPC, GH),
                             o[:, F + FH:2 * F].reshape(C, BPC, GH)], axis=2)
        tg = o[:, 2 * F:2 * F + NT]
        for bl in range(BPC):
            e = 0.0
            for g in range(1, G):
                d = (ls[:, bl, g] + K * sc) - (lv[:, bl, g - 1] + endcnt[g - 1] * sc)
                e += d.mean()
            part = lv[:, bl, G - 1] + endcnt[G - 1] * sc - e
            m = part.max()
            logz = np.log(np.exp(part - m).sum()) + m
            # tiles 2*bl, 2*bl+1 hold this sequence's bt rows
            tgt_e = tg[:, 2 * bl].sum() + tg[:, 2 * bl + 1].sum()
            loss[c * BPC + bl] = logz - tgt_e
    return loss.astype(np.float32)


def _run(inputs, trace=False):
    from concourse import bass_utils
    import ml_dtypes
    bf16 = ml_dtypes.bfloat16

    tokens = np.asarray(inputs["tokens"])
    target = np.asarray(inputs["target"])
    table = np.ascontiguousarray(np.asarray(inputs["state_table"], np.float32))
    trans = np.ascontiguousarray(np.asarray(inputs["trans_matrix"], np.float32))

    nc = _build()
    maps = _host_prep(tokens, target)
    tableb = np.ascontiguousarray(table.astype(bf16))
    transb = trans.astype(bf16)
    identb = np.eye(C, dtype=bf16)
    iota_row = np.broadcast_to(np.arange(C, dtype=np.float32), (128, C))
    for m in maps:
        ohp = m.pop("_ohp")
        tgtf = m.pop("_tgtf")
        m["ohpx"] = np.ascontiguousarray(
            np.concatenate([ohp, transb, identb], axis=1))
        m["transx"] = np.ascontiguousarray(np.concatenate(
            [trans, trans[C - 1:C, :].T, tgtf, iota_row,
             np.full((128, 1), -float(SBITS) * LN2, np.float32)], axis=1))
        m["tableb"] = tableb

    res = bass_utils.run_bass_kernel_spmd(
        nc, maps, core_ids=list(range(NCORES)), trace=trace)
    loss = _combine([r["out"] for r in res.results])
    return loss, res


def kernel(**inputs):
    loss, _ = _run(inputs, trace=False)
    return loss
